# revision 1
# baseline (speedup 1.0000x reference)
"""Trainium2 Bass kernel for nn_MultiHeadAttention_47579647705431.

Multi-head attention (8 heads, dim 512, seq 1024, batch 16) with:
  - shared key/query linear (key_query_same=True: q and k both use Wk/bk)
  - causal (or arbitrary block-structured) mask
  - SimpleKT zero_pad: attention row 0 zeroed => out[:, 0, :] = bo

Sharding: data-parallel over batch across 8 NeuronCores (2 batches/core).
Per-core device pipeline (all matmuls bf16, fp32 PSUM accumulation):
  host: cast+transpose q/k/v slices and weights to bf16 feature-major
  1. kp/qp = Wk.T-stationary projections -> feature-major [o, n] bf16
  2. vp   = token-major projection [n, o] bf16 with interleaved ones
           columns (stride-65 layout) for the denominator ones-row trick
  3. per (batch, head-pair, s-chunk): scores^T [t, s] via K=64 row-packed
     matmuls, additive -1e9 mask via identity-matmul accumulation on mixed
     blocks, exp on ACT (scale=1/8 folded), AV via [t, 65]-stationary
     matmuls accumulating [65, 512] PSUM whose row 64 is the softmax
     denominator; reciprocal + K=1 ones broadcast matmuls + DVE multiply
     normalize into concat^T bf16
  4. final projection (concat^T-stationary) -> [n, o] fp32 -> DRAM

The walrus build here supports ONE sync wait per instruction; Tile emits
more. legalize_waits() hoists extra waits onto same-engine NoOps.
"""

import os
from contextlib import ExitStack

import numpy as np
import ml_dtypes

import concourse.bass as bass
import concourse.mybir as mybir
import concourse.tile as tile
from concourse.bass_utils import run_bass_kernel_spmd

F32 = mybir.dt.float32
BF16 = mybir.dt.bfloat16
BF = ml_dtypes.bfloat16

B, S, D, H, DH = 16, 1024, 512, 8, 64
NCORES = 8
BL = B // NCORES          # batches per core
N = BL * S                # tokens per core
NB = S // 128             # 128-blocks per sequence (8)
HP = H // 2               # head pairs (= o-blocks of 128)
NEG = -1.0e9

LAST_SIM_NS = None
LAST_EXEC_NS = None


def legalize_waits(nc):
    """Split multi-wait instructions: keep one wait, hoist the rest onto
    preceding same-engine NoOps (this walrus encodes 1 wait/instruction)."""
    for f in nc.m.functions:
        for blk in f.blocks:
            il = blk.instructions
            i = 0
            while i < len(il):
                inst = il[i]
                si = inst.sync_info
                if si is not None and si.on_wait and len(si.on_wait) > 1:
                    waits = list(si.on_wait)
                    for j, w in enumerate(waits[:-1]):
                        nop = mybir.InstNoOp(
                            name=f"{inst.name}-hw{j}",
                            sync_info=mybir.SyncInfo(on_wait=[w], on_update=[]),
                            bass_nofuse=True,
                            engine=inst.engine,
                        )
                        il.insert(i, nop)
                        i += 1
                    si.on_wait = waits[-1:]
                i += 1


def _classify_mask(mask2d):
    """Classify 128x128 blocks of the [S, S] bool mask (query s, key t).

    Returns (status[j][i], patterns) in scores-transposed coords:
    j = key(t) block, i = query(s) block. status: -1 skip, -2 full,
    >=0 index into patterns (additive bf16 [t, s] blocks, 0 or NEG).
    """
    status = [[-1] * NB for _ in range(NB)]
    patterns = []
    pat_idx = {}
    for j in range(NB):
        for i in range(NB):
            blk = mask2d[i * 128:(i + 1) * 128, j * 128:(j + 1) * 128]  # [s, t]
            if blk.all():
                status[j][i] = -2
            elif not blk.any():
                status[j][i] = -1
            else:
                add = np.where(blk.T, 0.0, NEG).astype(BF)  # [t, s]
                key = add.tobytes()
                if key not in pat_idx:
                    pat_idx[key] = len(patterns)
                    patterns.append(add)
                status[j][i] = pat_idx[key]
    return status, patterns


def _plan_chunks(status, patterns):
    """Per (c, j): suffix run of non-skip query blocks within chunk c.

    Returns plan[c][j] = (w, mixes) where w = run width and mixes =
    [(col_offset_in_region, pattern_id), ...] for mixed blocks. Also
    first_j[c]. Asserts the suffix-nested structure the kernel relies on.
    """
    nch = S // 512
    plan = [[None] * NB for _ in range(nch)]
    first_j = [None] * nch
    for c in range(nch):
        i_lo, i_hi = 4 * c, 4 * c + 4
        prev_w = None
        for j in range(NB):
            sts = [status[j][i] for i in range(i_lo, i_hi)]
            nz = [k for k, s in enumerate(sts) if s != -1]
            if not nz:
                plan[c][j] = (0, [])
                continue
            # must be a contiguous suffix of the chunk
            if nz != list(range(nz[0], 4)):
                raise NotImplementedError("mask block structure not suffix-contiguous")
            w = 128 * len(nz)
            if prev_w is not None and w > prev_w:
                raise NotImplementedError("mask runs not nested over key blocks")
            prev_w = w
            mixes = [((k - nz[0]) * 128, sts[k]) for k in nz if sts[k] >= 0]
            plan[c][j] = (w, mixes)
            if first_j[c] is None:
                first_j[c] = j
    return plan, first_j


def _build(plan, first_j, nmix, has_bk, has_bv, has_bo):
    nc = bass.Bass()
    qt = nc.dram_tensor("qt", [128, 4, N], BF16, kind="ExternalInput")
    kt = nc.dram_tensor("kt", [128, 4, N], BF16, kind="ExternalInput")
    vt = nc.dram_tensor("vt", [128, 4, N], BF16, kind="ExternalInput")
    wkt = nc.dram_tensor("wkt", [128, 4, D], BF16, kind="ExternalInput")
    wvt = nc.dram_tensor("wvt", [128, 4, D], BF16, kind="ExternalInput")
    wot = nc.dram_tensor("wot", [128, 4, D], BF16, kind="ExternalInput")
    bk32 = nc.dram_tensor("bk32", [128, 4], F32, kind="ExternalInput")
    bvb = nc.dram_tensor("bvb", [1, D], BF16, kind="ExternalInput")
    bob = nc.dram_tensor("bob", [1, D], BF16, kind="ExternalInput")
    ident = nc.dram_tensor("ident", [128, 128], BF16, kind="ExternalInput")
    mixadd = nc.dram_tensor("mixadd", [max(nmix, 1), 128, 128], BF16,
                            kind="ExternalInput")
    out = nc.dram_tensor("out", [N, D], F32, kind="ExternalOutput")

    nch = S // 512

    with tile.TileContext(nc) as tc:
        with ExitStack() as ctx:
            sing = ctx.enter_context(tc.tile_pool(name="sing", bufs=1))
            expp = ctx.enter_context(tc.tile_pool(name="expp", bufs=5))
            rcp = ctx.enter_context(tc.tile_pool(name="rcp", bufs=3))
            outp = ctx.enter_context(tc.tile_pool(name="outp", bufs=4))
            stp = ctx.enter_context(tc.tile_pool(name="stp", bufs=2, space="PSUM"))
            avp = ctx.enter_context(tc.tile_pool(name="avp", bufs=2, space="PSUM"))
            shp = ctx.enter_context(tc.tile_pool(name="shp", bufs=2, space="PSUM"))

            # ---- input loads: critical-path first (kt/qt ch0 gate proj(0)),
            # cold constants (wot/ident/mixadd) last on the shared DMA device
            wkt_sb = sing.tile([128, 4, D], BF16)
            nc.sync.dma_start(out=wkt_sb, in_=wkt[:, :, :])
            wvt_sb = sing.tile([128, 4, D], BF16)
            nc.gpsimd.dma_start(out=wvt_sb, in_=wvt[:, :, :])
            kt_c, qt_c, vt_c = [], [], []
            for ch in range(4):
                csl = slice(ch * 512, ch * 512 + 512)
                t = sing.tile([128, 4, 512], BF16, tag=f"ktc{ch}")
                nc.sync.dma_start(out=t, in_=kt[:, :, csl])
                kt_c.append(t)
                t = sing.tile([128, 4, 512], BF16, tag=f"qtc{ch}")
                nc.scalar.dma_start(out=t, in_=qt[:, :, csl])
                qt_c.append(t)
            for ch in range(4):
                csl = slice(ch * 512, ch * 512 + 512)
                t = sing.tile([128, 4, 512], BF16, tag=f"vtc{ch}")
                nc.gpsimd.dma_start(out=t, in_=vt[:, :, csl])
                vt_c.append(t)
            ident_sb = sing.tile([128, 128], BF16)
            nc.sync.dma_start(out=ident_sb, in_=ident[:, :])
            mix_sb = sing.tile([128, max(nmix, 1), 128], BF16)
            nc.sync.dma_start(out=mix_sb, in_=mixadd.rearrange("m t s -> t m s"))
            wot_sb = sing.tile([128, 4, D], BF16)
            nc.scalar.dma_start(out=wot_sb, in_=wot[:, :, :])
            bk_sb = None
            if has_bk:
                bk_sb = sing.tile([128, 4], F32)
                nc.sync.dma_start(out=bk_sb, in_=bk32[:, :])
            bvb_sb = bob_sb = ones_k1 = None
            if has_bv or has_bo:
                ones_k1 = sing.tile([1, 128], BF16)
                nc.vector.memset(ones_k1, 1.0)
            if has_bv:
                bvb_sb = sing.tile([1, D], BF16)
                nc.sync.dma_start(out=bvb_sb, in_=bvb[:, :])
            if has_bo:
                bob_sb = sing.tile([1, D], BF16)
                nc.sync.dma_start(out=bob_sb, in_=bob[:, :])
            ones1 = sing.tile([1, 64], BF16)
            nc.vector.memset(ones1, 1.0)

            kp_sb = sing.tile([128, 4, N], BF16)
            qp_sb = sing.tile([128, 4, N], BF16)
            vp_sb = sing.tile([128, N // 128, 520], BF16)
            ct_sb = sing.tile([128, 4, N], BF16)

            # ---- phase functions (emitted interleaved for engine overlap) ----
            fillers = []

            def kq_group(ob, ch):
                    csl = slice(ch * 512, ch * 512 + 512)
                    psK = shp.tile([128, 512], F32, tag="sh")
                    psQ = shp.tile([128, 512], F32, tag="sh")
                    for db in range(4):
                        nc.tensor.matmul(
                            psK, wkt_sb[:, db, ob * 128:(ob + 1) * 128],
                            kt_c[ch][:, db, :], start=(db == 0), stop=(db == 3))
                    for db in range(4):
                        nc.tensor.matmul(
                            psQ, wkt_sb[:, db, ob * 128:(ob + 1) * 128],
                            qt_c[ch][:, db, :], start=(db == 0), stop=(db == 3))
                    if has_bk:
                        nc.scalar.add(kp_sb[:, ob, csl], psK, bk_sb[:, ob:ob + 1])
                        nc.vector.tensor_scalar_add(
                            qp_sb[:, ob, csl], psQ, bk_sb[:, ob:ob + 1])
                    elif (ob + ch) % 2 == 0:
                        nc.scalar.copy(kp_sb[:, ob, csl], psK)
                        nc.vector.tensor_copy(out=qp_sb[:, ob, csl], in_=psQ)
                    else:
                        nc.vector.tensor_copy(out=kp_sb[:, ob, csl], in_=psK)
                        nc.scalar.copy(qp_sb[:, ob, csl], psQ)

            def kq_proj(ob):
                for ch in range(N // 512):
                    kq_group(ob, ch)

            def v_proj(nt):
                psV = shp.tile([128, 512], F32, tag="sh")
                for db in range(4):
                    nc.tensor.matmul(
                        psV, vt_c[nt // 4][:, db, (nt % 4) * 128:(nt % 4) * 128 + 128],
                        wvt_sb[:, db, :], start=(db == 0),
                        stop=(db == 3 and not has_bv))
                if has_bv:
                    nc.tensor.matmul(psV, ones_k1, bvb_sb[0:1, :],
                                     start=False, stop=True)
                dst = vp_sb[:, nt, :].rearrange("p (h u) -> p h u", u=65)[:, :, 0:64]
                src = psV.rearrange("p (h u) -> p h u", u=64)
                if nt % 2 == 0:
                    nc.vector.tensor_copy(out=dst, in_=src)
                else:
                    nc.scalar.copy(dst, src)

            st_store = {}

            def attention_c(b, hp, c, next_start=None):
                h0, h1 = 2 * hp, 2 * hp + 1
                if True:
                    fj = first_j[c]
                    if fj is None:
                        return
                    av0 = avp.tile([65, 512], F32, tag="av")
                    av1 = avp.tile([65, 512], F32, tag="av")
                    js = [j for j in range(NB) if plan[c][j][0] > 0]
                    st_t = st_store.setdefault((b, hp, c), {})

                    def scores(j):
                        w, mixes = plan[c][j]
                        st = stp.tile([128, 1024], F32, tag="st")
                        st_t[j] = st
                        tsl = slice(b * S + j * 128, b * S + j * 128 + 128)
                        ssl = slice(b * S + c * 512 + 512 - w,
                                    b * S + c * 512 + 512)
                        nc.tensor.matmul(st[:, 512 - w:512],
                                         kp_sb[0:64, hp, tsl],
                                         qp_sb[0:64, hp, ssl],
                                         start=True, stop=not mixes)
                        nc.tensor.matmul(st[:, 512:512 + w],
                                         kp_sb[64:128, hp, tsl],
                                         qp_sb[64:128, hp, ssl],
                                         start=True, stop=not mixes)
                        for mi, (off, pid) in enumerate(mixes):
                            last = mi == len(mixes) - 1
                            o0 = 512 - w + off
                            nc.tensor.matmul(st[:, o0:o0 + 128], ident_sb,
                                             mix_sb[:, pid, :],
                                             start=False, stop=last)
                            nc.tensor.matmul(st[:, 512 + off:512 + off + 128],
                                             ident_sb, mix_sb[:, pid, :],
                                             start=False, stop=last)

                    if js[0] not in st_t:
                        scores(js[0])
                    for jx, j in enumerate(js):
                        w, _ = plan[c][j]
                        st = st_t.pop(j)
                        ex = expp.tile([128, 1024], BF16)
                        nc.scalar.activation(
                            ex[:, 0:2 * w], st[:, 512 - w:512 + w],
                            mybir.ActivationFunctionType.Exp, scale=0.125)
                        if jx + 1 < len(js):
                            scores(js[jx + 1])
                        elif next_start is not None:
                            next_start()
                        vrow = b * NB + j
                        nc.tensor.matmul(
                            av0[:, 512 - w:512],
                            vp_sb[:, vrow, 65 * h0:65 * h0 + 65],
                            ex[:, 0:w], start=(j == fj),
                            stop=(jx == len(js) - 1))
                        nc.tensor.matmul(
                            av1[:, 512 - w:512],
                            vp_sb[:, vrow, 65 * h1:65 * h1 + 65],
                            ex[:, w:2 * w], start=(j == fj),
                            stop=(jx == len(js) - 1))
                        if fillers and jx % 2 == 1:
                            # fill PE only in each block's second half, where
                            # ACT's exp lag has accumulated; spreads the
                            # filler supply across many block boundaries
                            fillers.pop(0)()

                    rc = rcp.tile([1, 1024], BF16)
                    with nc.allow_low_precision(reason="softmax recip bf16"):
                        nc.vector.reciprocal(out=rc[0:1, 0:512],
                                             in_=av0[64:65, :])
                        nc.vector.reciprocal(out=rc[0:1, 512:1024],
                                             in_=av1[64:65, :])

                    def epilogue(rc=rc, av0=av0, av1=av1, b=b, hp=hp, c=c):
                        # deferred into the next block's filler slots so the
                        # PE bc matmuls don't stall on the reciprocal latency
                        bc = shp.tile([128, 512], F32, tag="sh")
                        nc.tensor.matmul(bc[0:64, :], ones1, rc[0:1, 0:512],
                                         start=True, stop=True)
                        nc.tensor.matmul(bc[64:128, :], ones1,
                                         rc[0:1, 512:1024],
                                         start=True, stop=True,
                                         tile_position=(0, 64))
                        bcs = rcp.tile([128, 512], BF16, tag="bcs")
                        nc.vector.tensor_copy(out=bcs, in_=bc)
                        osl = slice(b * S + c * 512, b * S + c * 512 + 512)
                        nc.vector.tensor_mul(ct_sb[0:64, hp, osl],
                                             av0[0:64, :], bcs[0:64, :])
                        nc.vector.tensor_mul(ct_sb[64:128, hp, osl],
                                             av1[0:64, :], bcs[64:128, :])

                    fillers.insert(0, epilogue)

            def flush_fillers():
                while fillers:
                    fillers.pop(0)()

            def final_group(b, nt):
                if True:
                    gnt = b * NB + nt
                    psO = shp.tile([128, 512], F32, tag="sh")
                    for hp in range(4):
                        nc.tensor.matmul(
                            psO, ct_sb[:, hp, gnt * 128:(gnt + 1) * 128],
                            wot_sb[:, hp, :], start=(hp == 0),
                            stop=(hp == 3 and not has_bo))
                    if has_bo:
                        nc.tensor.matmul(psO, ones_k1, bob_sb[0:1, :],
                                         start=False, stop=True)
                    ot = outp.tile([128, 512], F32)
                    if nt % 2 == 0:
                        nc.vector.tensor_copy(out=ot, in_=psO)
                        nc.scalar.dma_start(
                            out=out[gnt * 128:(gnt + 1) * 128, :], in_=ot)
                    else:
                        nc.scalar.copy(ot, psO)
                        nc.sync.dma_start(
                            out=out[gnt * 128:(gnt + 1) * 128, :], in_=ot)

            # ones columns of vp
            for nt in range(N // 128):
                nc.vector.memset(
                    vp_sb[:, nt, :].rearrange("p (h u) -> p h u", u=65)[:, :, 64:65],
                    1.0)

            # interleaved emission: ACT exp work starts as early as possible;
            # attention split by s-chunk so final-projection halves overlap
            kq_proj(0)
            for nt in range(NB):
                v_proj(nt)
            import functools
            for nt in range(NB, N // 128):
                v_proj(nt)
            for ob in (1, 2, 3):
                for ch in range(N // 512):
                    fillers.append(functools.partial(kq_group, ob, ch))
            seq = [(0, 0, 1), (1, 0, 1), (0, 1, 1), (1, 1, 1), (0, 2, 1),
                   (1, 2, 1), (0, 3, 1), (1, 3, 1), (0, 0, 0), (1, 0, 0),
                   (0, 1, 0), (1, 1, 0), (0, 2, 0), (1, 2, 0), (0, 3, 0),
                   (1, 3, 0)]

            def make_next_start(nb_, nhp_, nc_):
                def _start():
                    # emit the next block's first scores inside this block so
                    # PE keeps ACT fed across the block boundary
                    h0_, h1_ = 2 * nhp_, 2 * nhp_ + 1
                    js_ = [j for j in range(NB) if plan[nc_][j][0] > 0]
                    if not js_:
                        return
                    j = js_[0]
                    w, mixes = plan[nc_][j]
                    st = stp.tile([128, 1024], F32, tag="st")
                    st_store.setdefault((nb_, nhp_, nc_), {})[j] = st
                    tsl = slice(nb_ * S + j * 128, nb_ * S + j * 128 + 128)
                    ssl = slice(nb_ * S + nc_ * 512 + 512 - w,
                                nb_ * S + nc_ * 512 + 512)
                    nc.tensor.matmul(st[:, 512 - w:512],
                                     kp_sb[0:64, nhp_, tsl],
                                     qp_sb[0:64, nhp_, ssl],
                                     start=True, stop=not mixes)
                    nc.tensor.matmul(st[:, 512:512 + w],
                                     kp_sb[64:128, nhp_, tsl],
                                     qp_sb[64:128, nhp_, ssl],
                                     start=True, stop=not mixes)
                    for mi, (off, pid) in enumerate(mixes):
                        last = mi == len(mixes) - 1
                        o0 = 512 - w + off
                        nc.tensor.matmul(st[:, o0:o0 + 128], ident_sb,
                                         mix_sb[:, pid, :],
                                         start=False, stop=last)
                        nc.tensor.matmul(st[:, 512 + off:512 + off + 128],
                                         ident_sb, mix_sb[:, pid, :],
                                         start=False, stop=last)
                return _start

            for bi, (b_, hp_, c_) in enumerate(seq):
                nxt = make_next_start(*seq[bi + 1]) if bi + 1 < len(seq) else None
                attention_c(b_, hp_, c_, next_start=nxt)
                if (b_, hp_, c_) == (0, 3, 1):
                    for nt_ in range(4, 8):
                        fillers.append(functools.partial(final_group, 0, nt_))
                elif (b_, hp_, c_) == (1, 3, 1):
                    for nt_ in range(4, 8):
                        fillers.append(functools.partial(final_group, 1, nt_))
                elif (b_, hp_, c_) == (0, 3, 0):
                    for nt_ in range(0, 4):
                        fillers.append(functools.partial(final_group, 0, nt_))
                elif (b_, hp_, c_) == (1, 3, 0):
                    for nt_ in range(0, 4):
                        fillers.append(functools.partial(final_group, 1, nt_))
            flush_fillers()

    return nc


_prog_cache = {}


def kernel(q, k, v, mask, zero_pad, Wk, bk, Wv, bv, Wo, bo):
    global LAST_SIM_NS, LAST_EXEC_NS
    q = np.asarray(q, dtype=np.float32)
    k = np.asarray(k, dtype=np.float32)
    v = np.asarray(v, dtype=np.float32)
    Wk = np.asarray(Wk, dtype=np.float32)
    Wv = np.asarray(Wv, dtype=np.float32)
    Wo = np.asarray(Wo, dtype=np.float32)
    bk = np.asarray(bk, dtype=np.float32).reshape(D)
    bv = np.asarray(bv, dtype=np.float32).reshape(D)
    bo = np.asarray(bo, dtype=np.float32).reshape(D)
    mask2d = np.asarray(mask).reshape(S, S).astype(bool)
    zp = int(np.asarray(zero_pad))

    status, patterns = _classify_mask(mask2d)
    plan, first_j = _plan_chunks(status, patterns)
    nmix = len(patterns)
    has_bk = bool(np.any(bk))
    has_bv = bool(np.any(bv))
    has_bo = bool(np.any(bo))

    sig = (tuple(tuple(r) for r in status), nmix, has_bk, has_bv, has_bo)
    if sig not in _prog_cache:
        nc_new = _build(plan, first_j, nmix, has_bk, has_bv, has_bo)
        legalize_waits(nc_new)   # hardware-only pass (sim runs pre-legalized)
        _prog_cache[sig] = nc_new
    nc = _prog_cache[sig]

    def _sbuf_layout(wt):
        # [D, X] -> [128, 4, X]: row d = a*128+p  ->  [p, a, :]
        return np.ascontiguousarray(wt.reshape(4, 128, -1).transpose(1, 0, 2))

    wkt = _sbuf_layout(Wk.T.astype(BF))
    wvt = _sbuf_layout(Wv.T.astype(BF))
    wot = _sbuf_layout(Wo.T.astype(BF))
    bk32 = np.ascontiguousarray(bk.reshape(4, 128).T).astype(np.float32)
    bvb = bv.reshape(1, D).astype(BF)
    bob = bo.reshape(1, D).astype(BF)
    ident = np.eye(128, dtype=BF)
    mixadd = (np.stack(patterns) if patterns
              else np.zeros((1, 128, 128), np.float32)).astype(BF)

    common = dict(wkt=wkt, wvt=wvt, wot=wot, bk32=bk32, bvb=bvb, bob=bob,
                  ident=ident, mixadd=mixadd)
    in_maps = []
    for ci in range(NCORES):
        sl = slice(ci * BL, (ci + 1) * BL)
        in_maps.append(dict(
            qt=_sbuf_layout(q[sl].reshape(N, D).T.astype(BF)),
            kt=_sbuf_layout(k[sl].reshape(N, D).T.astype(BF)),
            vt=_sbuf_layout(v[sl].reshape(N, D).T.astype(BF)),
            **common))

    if os.environ.get("BASS_KERNEL_SIM_TIME"):
        from concourse.timeline_sim import TimelineSim
        LAST_SIM_NS = TimelineSim(nc).simulate()

    res = run_bass_kernel_spmd(nc, in_maps, list(range(NCORES)))
    LAST_EXEC_NS = res.exec_time_ns

    outs = [res.results[ci]["out"].reshape(BL, S, D) for ci in range(NCORES)]
    full = np.concatenate(outs, axis=0)
    if zp:
        full[:, 0, :] = bo
    return full



# revision 7
# speedup vs baseline: 1.1081x; 1.1081x over previous
"""Trainium2 Bass kernel for nn_MultiHeadAttention_47579647705431.

Multi-head attention (8 heads, dim 512, seq 1024, batch 16) with:
  - shared key/query linear (key_query_same=True: q and k both use Wk/bk)
  - causal (or arbitrary block-structured) mask
  - SimpleKT zero_pad: attention row 0 zeroed => out[:, 0, :] = bo

Sharding: data-parallel over batch across 8 NeuronCores (2 batches/core).

Per-core pipeline (all matmuls bf16, fp32 PSUM):
  1. kp/qp = Wk.T-stationary projections -> feature-major [o, n] bf16
  2. vp    = token-major projection [n, o] bf16 with interleaved ones
             columns (stride-65) providing the softmax denominator column
  3. per (b, hp, c): scores^T st [t, s] via K=64 row-packed matmuls;
     exp on ACT (scale 1/8 folded, several j-blocks packed per
     activation); causal/diagonal masking as a 0/1 multiply on DVE
  4. AV with SWAPPED operands: stationary = ex [t, s-block], moving =
     vp [t, 65] -> av PSUM [s, 2, 65] per (b, i, hp).  Cost = 65 free
     columns per (i, j, head) instead of 512 -- half the PE cycles of
     the stationary-vp form, and the denominator lands per-partition so
     normalization is a DVE reciprocal + broadcast multiply (no PE
     broadcast matmuls, no mask identity matmuls).
  5. ct_t token-major [s, 512] per (b, i) -> feature-major ct_i
     [128, 4, 128] via one XBAR dma_start_transpose
  6. out projection per (b, i) (ct_i-stationary) -> [128, 512] f32 -> DRAM

The walrus build here supports ONE sync wait per instruction; Tile emits
more. legalize_waits() hoists extra waits onto same-engine NoOps.
"""

import os
from contextlib import ExitStack

import numpy as np
import ml_dtypes

import concourse.bass as bass
import concourse.mybir as mybir
import concourse.tile as tile
from concourse.bass_utils import run_bass_kernel_spmd

F32 = mybir.dt.float32
BF16 = mybir.dt.bfloat16
BF = ml_dtypes.bfloat16

B, S, D, H, DH = 16, 1024, 512, 8, 64
NCORES = 8
BL = B // NCORES          # batches per core
N = BL * S                # tokens per core
NB = S // 128             # 128-blocks per sequence (8)
HP = H // 2               # head pairs (= o-blocks of 128)
NCH = S // 512            # 512-chunks per sequence (2)

LAST_SIM_NS = None
LAST_EXEC_NS = None


def legalize_waits(nc):
    """Split multi-wait instructions: keep one wait, hoist the rest onto
    preceding same-engine NoOps (this walrus encodes 1 wait/instruction)."""
    for f in nc.m.functions:
        for blk in f.blocks:
            il = blk.instructions
            i = 0
            while i < len(il):
                inst = il[i]
                si = inst.sync_info
                if si is not None and si.on_wait and len(si.on_wait) > 1:
                    waits = list(si.on_wait)
                    for j, w in enumerate(waits[:-1]):
                        nop = mybir.InstNoOp(
                            name=f"{inst.name}-hw{j}",
                            sync_info=mybir.SyncInfo(on_wait=[w], on_update=[]),
                            bass_nofuse=True,
                            engine=inst.engine,
                        )
                        il.insert(i, nop)
                        i += 1
                    si.on_wait = waits[-1:]
                i += 1


def _classify_mask(mask2d):
    """Classify 128x128 blocks of the [S, S] bool mask (query s, key t).

    Returns (status[j][i], patterns) in scores-transposed coords:
    j = key(t) block, i = query(s) block. status: -1 skip, -2 full,
    >=0 index into patterns (multiplicative bf16 0/1 [t, s] blocks).
    """
    status = [[-1] * NB for _ in range(NB)]
    patterns = []
    pat_idx = {}
    for j in range(NB):
        for i in range(NB):
            blk = mask2d[i * 128:(i + 1) * 128, j * 128:(j + 1) * 128]  # [s, t]
            if blk.all():
                status[j][i] = -2
            elif not blk.any():
                status[j][i] = -1
            else:
                mul = np.where(blk.T, 1.0, 0.0).astype(BF)  # [t, s]
                key = mul.tobytes()
                if key not in pat_idx:
                    pat_idx[key] = len(patterns)
                    patterns.append(mul)
                status[j][i] = pat_idx[key]
    return status, patterns


def _plan_chunks(status, patterns):
    """Per (c, j): suffix run of non-skip query blocks within chunk c.

    Returns plan[c][j] = (w, mixes) where w = run width and mixes =
    [(col_offset_in_region, pattern_id), ...] for mixed blocks. Also
    first_j[c]. Asserts the suffix-nested structure the kernel relies on.
    """
    plan = [[None] * NB for _ in range(NCH)]
    first_j = [None] * NCH
    for c in range(NCH):
        i_lo, i_hi = 4 * c, 4 * c + 4
        prev_w = None
        for j in range(NB):
            sts = [status[j][i] for i in range(i_lo, i_hi)]
            nz = [k for k, s in enumerate(sts) if s != -1]
            if not nz:
                plan[c][j] = (0, [])
                continue
            # must be a contiguous suffix of the chunk
            if nz != list(range(nz[0], 4)):
                raise NotImplementedError("mask block structure not suffix-contiguous")
            w = 128 * len(nz)
            if prev_w is not None and w > prev_w:
                raise NotImplementedError("mask runs not nested over key blocks")
            prev_w = w
            mixes = [((k - nz[0]) * 128, sts[k]) for k in nz if sts[k] >= 0]
            plan[c][j] = (w, mixes)
            if first_j[c] is None:
                first_j[c] = j
    return plan, first_j


def _pack_js(plan, c):
    """Greedy-pack consecutive j runs so one st tile / one exp covers
    several j blocks.  Each pack's total 2w must fit 1024 f32 (4KB)."""
    js = [j for j in range(NB) if plan[c][j][0] > 0]
    packs = []
    cur, cur_sz = [], 0
    for j in js:
        sz = 2 * plan[c][j][0]
        if cur and cur_sz + sz > 1024:
            packs.append(cur)
            cur, cur_sz = [], 0
        cur.append(j)
        cur_sz += sz
    if cur:
        packs.append(cur)
    return packs


def _build(plan, first_j, nmix, has_bk, has_bv, has_bo):
    nc = bass.Bass()
    qt = nc.dram_tensor("qt", [128, 4, N], BF16, kind="ExternalInput")
    kt = nc.dram_tensor("kt", [128, 4, N], BF16, kind="ExternalInput")
    vt = nc.dram_tensor("vt", [128, 4, N], BF16, kind="ExternalInput")
    wkt = nc.dram_tensor("wkt", [128, 4, D], BF16, kind="ExternalInput")
    wvt = nc.dram_tensor("wvt", [128, 4, D], BF16, kind="ExternalInput")
    wot = nc.dram_tensor("wot", [128, 4, D], BF16, kind="ExternalInput")
    bk32 = nc.dram_tensor("bk32", [128, 4], F32, kind="ExternalInput")
    bvb = nc.dram_tensor("bvb", [1, D], BF16, kind="ExternalInput")
    bob = nc.dram_tensor("bob", [1, D], BF16, kind="ExternalInput")
    mixmul = nc.dram_tensor("mixmul", [max(nmix, 1), 128, 128], BF16,
                            kind="ExternalInput")
    out = nc.dram_tensor("out", [N, D], F32, kind="ExternalOutput")

    with tile.TileContext(nc) as tc:
        with ExitStack() as ctx:
            sing = ctx.enter_context(tc.tile_pool(name="sing", bufs=1))
            expp = ctx.enter_context(tc.tile_pool(name="expp", bufs=12))
            rcp = ctx.enter_context(tc.tile_pool(name="rcp", bufs=4))
            ctp = ctx.enter_context(tc.tile_pool(name="ctp", bufs=6))
            cti = ctx.enter_context(tc.tile_pool(name="cti", bufs=4))
            outp = ctx.enter_context(tc.tile_pool(name="outp", bufs=4))
            stp = ctx.enter_context(tc.tile_pool(name="stp", bufs=2, space="PSUM"))
            avp = ctx.enter_context(tc.tile_pool(name="avp", bufs=2, space="PSUM"))
            shp = ctx.enter_context(tc.tile_pool(name="shp", bufs=2, space="PSUM"))

            # ---- input loads: critical-path first.  Attention-phase DMAs
            # go on sync; startup loads spread over scalar/sync/gpsimd.
            wkt_sb = sing.tile([128, 4, D], BF16)
            nc.sync.dma_start(out=wkt_sb, in_=wkt[:, :, :])
            wvt_sb = sing.tile([128, 4, D], BF16)
            nc.gpsimd.dma_start(out=wvt_sb, in_=wvt[:, :, :])
            kt_c, qt_c, vt_c = [], [], []
            for ch in range(4):
                csl = slice(ch * 512, ch * 512 + 512)
                t = sing.tile([128, 4, 512], BF16, tag=f"ktc{ch}")
                nc.sync.dma_start(out=t, in_=kt[:, :, csl])
                kt_c.append(t)
                t = sing.tile([128, 4, 512], BF16, tag=f"qtc{ch}")
                nc.scalar.dma_start(out=t, in_=qt[:, :, csl])
                qt_c.append(t)
            for ch in range(4):
                csl = slice(ch * 512, ch * 512 + 512)
                t = sing.tile([128, 4, 512], BF16, tag=f"vtc{ch}")
                nc.gpsimd.dma_start(out=t, in_=vt[:, :, csl])
                vt_c.append(t)
            mix_sb = sing.tile([128, max(nmix, 1), 128], BF16)
            nc.scalar.dma_start(out=mix_sb, in_=mixmul.rearrange("m t s -> t m s"))
            wot_sb = sing.tile([128, 4, D], BF16)
            nc.scalar.dma_start(out=wot_sb, in_=wot[:, :, :])
            bk_sb = None
            if has_bk:
                bk_sb = sing.tile([128, 4], F32)
                nc.sync.dma_start(out=bk_sb, in_=bk32[:, :])
            bvb_sb = bob_sb = ones_k1 = None
            if has_bv or has_bo:
                ones_k1 = sing.tile([1, 128], BF16)
                nc.vector.memset(ones_k1, 1.0)
            if has_bv:
                bvb_sb = sing.tile([1, D], BF16)
                nc.sync.dma_start(out=bvb_sb, in_=bvb[:, :])
            if has_bo:
                bob_sb = sing.tile([1, D], BF16)
                nc.sync.dma_start(out=bob_sb, in_=bob[:, :])

            kp_sb = sing.tile([128, 4, N], BF16)
            qp_sb = sing.tile([128, 4, N], BF16)
            vp_sb = sing.tile([128, N // 128, 520], BF16)

            # ones columns of vp (denominator trick)
            for nt in range(N // 128):
                nc.vector.memset(
                    vp_sb[:, nt, :].rearrange("p (h u) -> p h u", u=65)[:, :, 64:65],
                    1.0)

            fillers = []
            copy_rr = [0]

            def psum_copy(dst, src):
                # round-robin PSUM->SBUF copies over DVE (3/4) and ACT (1/4)
                # (GPSIMD cannot access PSUM)
                r = copy_rr[0] % 4
                copy_rr[0] += 1
                if r == 1:
                    nc.scalar.copy(dst, src)
                else:
                    nc.vector.tensor_copy(out=dst, in_=src)

            def kq_group(ob, ch):
                csl = slice(ch * 512, ch * 512 + 512)
                psK = shp.tile([128, 512], F32, tag="sh")
                psQ = shp.tile([128, 512], F32, tag="sh")
                for db in range(4):
                    nc.tensor.matmul(
                        psK, wkt_sb[:, db, ob * 128:(ob + 1) * 128],
                        kt_c[ch][:, db, :], start=(db == 0), stop=(db == 3))
                for db in range(4):
                    nc.tensor.matmul(
                        psQ, wkt_sb[:, db, ob * 128:(ob + 1) * 128],
                        qt_c[ch][:, db, :], start=(db == 0), stop=(db == 3))
                if has_bk:
                    nc.scalar.add(kp_sb[:, ob, csl], psK, bk_sb[:, ob:ob + 1])
                    nc.vector.tensor_scalar_add(
                        qp_sb[:, ob, csl], psQ, bk_sb[:, ob:ob + 1])
                else:
                    psum_copy(kp_sb[:, ob, csl], psK)
                    psum_copy(qp_sb[:, ob, csl], psQ)

            def v_proj(nt):
                psV = shp.tile([128, 512], F32, tag="sh")
                for db in range(4):
                    nc.tensor.matmul(
                        psV, vt_c[nt // 4][:, db, (nt % 4) * 128:(nt % 4) * 128 + 128],
                        wvt_sb[:, db, :], start=(db == 0),
                        stop=(db == 3 and not has_bv))
                if has_bv:
                    nc.tensor.matmul(psV, ones_k1, bvb_sb[0:1, :],
                                     start=False, stop=True)
                dst = vp_sb[:, nt, :].rearrange("p (h u) -> p h u", u=65)[:, :, 0:64]
                src = psV.rearrange("p (h u) -> p h u", u=64)
                psum_copy(dst, src)

            def pop_filler(k=1):
                for _ in range(k):
                    if fillers:
                        fillers.pop(0)()

            ct_t_tiles = {}

            def attention_hp(b, hp, c, last_hp):
                """Scores+exp+mask for all j of (b, hp, c), then AV per i."""
                packs = _pack_js(plan, c)
                if not packs:
                    return
                h0, h1 = 2 * hp, 2 * hp + 1
                ex_t = {}     # j -> (ex_tile, col_off, w)

                for pack in packs:
                    # PSUM bank rule: each matmul output must stay inside one
                    # 2KB bank.  h0 segments stack downward from col 512
                    # (bank 0), h1 segments upward from col 512 (bank 1); the
                    # exp covers the contiguous union [512-tw, 512+tw).
                    tw = sum(plan[c][j][0] for j in pack)
                    st = stp.tile([128, 1024], F32, tag="st")
                    ex = expp.tile([128, 1024], BF16, tag="ex")
                    pre = 0
                    for j in pack:
                        w, _ = plan[c][j]
                        tsl = slice(b * S + j * 128, b * S + j * 128 + 128)
                        ssl = slice(b * S + c * 512 + 512 - w,
                                    b * S + c * 512 + 512)
                        h0s = 512 - pre - w
                        h1s = 512 + pre
                        nc.tensor.matmul(st[:, h0s:h0s + w],
                                         kp_sb[0:64, hp, tsl],
                                         qp_sb[0:64, hp, ssl],
                                         start=True, stop=True)
                        nc.tensor.matmul(st[:, h1s:h1s + w],
                                         kp_sb[64:128, hp, tsl],
                                         qp_sb[64:128, hp, ssl],
                                         start=True, stop=True)
                        ex_t[j] = (ex, h0s, h1s, w)
                        pre += w
                    nc.scalar.activation(
                        ex[:, 512 - tw:512 + tw], st[:, 512 - tw:512 + tw],
                        mybir.ActivationFunctionType.Exp, scale=0.125)
                    # 0/1 mask multiply for mixed blocks (per head half)
                    for j in pack:
                        w, mixes = plan[c][j]
                        _, h0s, h1s, _ = ex_t[j]
                        for moff, pid in mixes:
                            for hs in (h0s, h1s):
                                sl = ex[:, hs + moff:hs + moff + 128]
                                nc.vector.tensor_mul(
                                    sl, sl, mix_sb[:, pid, :])
                    pop_filler(1)

                # AV, i-major; swapped operands
                for i in range(4 * c, 4 * c + 4):
                    js_i = []
                    for j, (ex, h0s, h1s, w) in ex_t.items():
                        i_start = 4 * c + 4 - w // 128
                        if i >= i_start:
                            o = (i - i_start) * 128
                            js_i.append((j, ex, (h0s + o, h1s + o)))
                    if not js_i:
                        continue
                    av = avp.tile([128, 2, 65], F32, tag="av")
                    nmm = len(js_i) * 2
                    mi = 0
                    for j, ex, hss in js_i:
                        vrow = b * NB + j
                        for h in range(2):
                            # single accumulation group per av tile: PSUM
                            # zeroing is bank-granular (start marks the whole
                            # bank pending-zero; first write to each address
                            # assigns, later writes accumulate)
                            nc.tensor.matmul(
                                av[:, h, :],
                                ex[:, hss[h]:hss[h] + 128],
                                vp_sb[:, vrow,
                                      65 * (h0 + h):65 * (h0 + h) + 65],
                                start=(mi == 0), stop=(mi == nmm - 1),
                                skip_group_check=True)
                            mi += 1
                    # normalize: per-partition reciprocal + broadcast mul
                    key = (b, i)
                    if key not in ct_t_tiles:
                        ct_t_tiles[key] = ctp.tile([128, 512], BF16,
                                                   name=f"ctt{b}_{i}",
                                                   tag=f"ctt{i % 4}")
                    ct_t = ct_t_tiles[key]
                    rc = rcp.tile([128, 2], BF16, tag="rc")
                    with nc.allow_low_precision(reason="softmax recip bf16"):
                        nc.vector.reciprocal(out=rc, in_=av[:, :, 64])
                    dst = ct_t[:, 128 * hp:128 * hp + 128].rearrange(
                        "p (h w) -> p h w", h=2)
                    nc.vector.tensor_mul(
                        dst, av[:, :, 0:64],
                        rc[:, :, None].broadcast_to([128, 2, 64]))
                    if last_hp:
                        finish_block(b, i, ct_t)
                    pop_filler(1)

            def finish_block(b, i, ct_t):
                """transpose ct_t -> feature-major, then queue out-proj."""
                ct_i = cti.tile([128, 4, 128], BF16, tag="cti")
                nc.sync.dma_start_transpose(ct_i[:, :, :], ct_t[:, :])
                del ct_t_tiles[(b, i)]

                def final(b=b, i=i, ct_i=ct_i):
                    psO = shp.tile([128, 512], F32, tag="sh")
                    for db in range(4):
                        nc.tensor.matmul(
                            psO, ct_i[:, db, :], wot_sb[:, db, :],
                            start=(db == 0), stop=(db == 3 and not has_bo))
                    if has_bo:
                        nc.tensor.matmul(psO, ones_k1, bob_sb[0:1, :],
                                         start=False, stop=True)
                    ot = outp.tile([128, 512], F32)
                    psum_copy(ot, psO)
                    row = b * S + i * 128
                    nc.sync.dma_start(out=out[row:row + 128, :], in_=ot)

                fillers.append(final)

            # ---- emission schedule ----
            # upfront: kq proj ob0 (all chunks) + v proj for b0
            for ch in range(4):
                kq_group(0, ch)
            for nt in range(NB):
                v_proj(nt)
            import functools
            for ob in (1, 2, 3):
                for ch in range(4):
                    fillers.append(functools.partial(kq_group, ob, ch))
            for nt in range(NB, N // 128):
                fillers.append(functools.partial(v_proj, nt))

            seq = []
            for c in (1, 0) if NCH == 2 else range(NCH - 1, -1, -1):
                for b in range(BL):
                    for hp in range(4):
                        seq.append((b, hp, c))
            for (b, hp, c) in seq:
                attention_hp(b, hp, c, last_hp=(hp == 3))
            while fillers:
                pop_filler(1)

    return nc


_prog_cache = {}


def kernel(q, k, v, mask, zero_pad, Wk, bk, Wv, bv, Wo, bo):
    global LAST_SIM_NS, LAST_EXEC_NS
    q = np.asarray(q, dtype=np.float32)
    k = np.asarray(k, dtype=np.float32)
    v = np.asarray(v, dtype=np.float32)
    Wk = np.asarray(Wk, dtype=np.float32)
    Wv = np.asarray(Wv, dtype=np.float32)
    Wo = np.asarray(Wo, dtype=np.float32)
    bk = np.asarray(bk, dtype=np.float32).reshape(D)
    bv = np.asarray(bv, dtype=np.float32).reshape(D)
    bo = np.asarray(bo, dtype=np.float32).reshape(D)
    mask2d = np.asarray(mask).reshape(S, S).astype(bool)
    zp = int(np.asarray(zero_pad))

    status, patterns = _classify_mask(mask2d)
    plan, first_j = _plan_chunks(status, patterns)
    nmix = len(patterns)
    has_bk = bool(np.any(bk))
    has_bv = bool(np.any(bv))
    has_bo = bool(np.any(bo))

    sig = (tuple(tuple(r) for r in status), nmix, has_bk, has_bv, has_bo)
    if sig not in _prog_cache:
        nc_new = _build(plan, first_j, nmix, has_bk, has_bv, has_bo)
        legalize_waits(nc_new)   # hardware-only pass (sim runs pre-legalized)
        _prog_cache[sig] = nc_new
    nc = _prog_cache[sig]

    def _sbuf_layout(wt):
        # [D, X] -> [128, 4, X]: row d = a*128+p  ->  [p, a, :]
        return np.ascontiguousarray(wt.reshape(4, 128, -1).transpose(1, 0, 2))

    wkt = _sbuf_layout(Wk.T.astype(BF))
    wvt = _sbuf_layout(Wv.T.astype(BF))
    wot = _sbuf_layout(Wo.T.astype(BF))
    bk32 = np.ascontiguousarray(bk.reshape(4, 128).T).astype(np.float32)
    bvb = bv.reshape(1, D).astype(BF)
    bob = bo.reshape(1, D).astype(BF)
    mixmul = (np.stack(patterns) if patterns
              else np.zeros((1, 128, 128), np.float32)).astype(BF)

    common = dict(wkt=wkt, wvt=wvt, wot=wot, bk32=bk32, bvb=bvb, bob=bob,
                  mixmul=mixmul)
    in_maps = []
    for ci in range(NCORES):
        sl = slice(ci * BL, (ci + 1) * BL)
        in_maps.append(dict(
            qt=_sbuf_layout(q[sl].reshape(N, D).T.astype(BF)),
            kt=_sbuf_layout(k[sl].reshape(N, D).T.astype(BF)),
            vt=_sbuf_layout(v[sl].reshape(N, D).T.astype(BF)),
            **common))

    if os.environ.get("BASS_KERNEL_SIM_TIME"):
        from concourse.timeline_sim import TimelineSim
        LAST_SIM_NS = TimelineSim(nc).simulate()

    res = run_bass_kernel_spmd(nc, in_maps, list(range(NCORES)))
    LAST_EXEC_NS = res.exec_time_ns

    outs = [res.results[ci]["out"].reshape(BL, S, D) for ci in range(NCORES)]
    full = np.concatenate(outs, axis=0)
    if zp:
        full[:, 0, :] = bo
    return full


# revision 47
# speedup vs baseline: 1.2235x; 1.1041x over previous
"""Trainium2 Bass kernel for nn_MultiHeadAttention_47579647705431.

Multi-head attention (8 heads, dim 512, seq 1024, batch 16) with:
  - shared key/query linear (key_query_same=True: q and k both use Wk/bk)
  - causal (or arbitrary block-structured) mask
  - SimpleKT zero_pad: attention row 0 zeroed => out[:, 0, :] = bo

Sharding: data-parallel over batch across 8 NeuronCores (2 batches/core).

Per-core pipeline (all matmuls bf16, fp32 PSUM):
  1. kp/qp = Wk.T-stationary projections -> feature-major [o, n] bf16
  2. vp    = token-major projection [n, o] bf16 with interleaved ones
             columns (stride-65) providing the softmax denominator column
  3. per (b, hp, c): scores^T st [t, s] via K=64 row-packed matmuls;
     exp on ACT (scale 1/8 folded, several j-blocks packed per
     activation); causal/diagonal masking as a 0/1 multiply on DVE
  4. AV with SWAPPED operands: stationary = ex [t, s-block], moving =
     vp [t, 65] -> av PSUM [s, 2, 65] per (b, i, hp).  Cost = 65 free
     columns per (i, j, head) instead of 512 -- half the PE cycles of
     the stationary-vp form, and the denominator lands per-partition so
     normalization is a DVE reciprocal + broadcast multiply (no PE
     broadcast matmuls, no mask identity matmuls).
  5. ct_t token-major [s, 512] per (b, i) -> feature-major ct_i
     [128, 4, 128] via one XBAR dma_start_transpose
  6. out projection per (b, i) (ct_i-stationary) -> [128, 512] f32 -> DRAM

The walrus build here supports ONE sync wait per instruction; Tile emits
more. legalize_waits() hoists extra waits onto same-engine NoOps.
"""

import os
from contextlib import ExitStack

import numpy as np
import ml_dtypes

import concourse.bass as bass
import concourse.mybir as mybir
import concourse.tile as tile
from concourse.bass_utils import run_bass_kernel_spmd

F32 = mybir.dt.float32
BF16 = mybir.dt.bfloat16
BF = ml_dtypes.bfloat16

B, S, D, H, DH = 16, 1024, 512, 8, 64
NCORES = 8
BL = B // NCORES          # batches per core
N = BL * S                # tokens per core
NB = S // 128             # 128-blocks per sequence (8)
HP = H // 2               # head pairs (= o-blocks of 128)
NCH = S // 512            # 512-chunks per sequence (2)

LAST_SIM_NS = None
LAST_EXEC_NS = None


def legalize_waits(nc):
    """Split multi-wait instructions: keep one wait, hoist the rest onto
    preceding same-engine NoOps (this walrus encodes 1 wait/instruction)."""
    for f in nc.m.functions:
        for blk in f.blocks:
            il = blk.instructions
            i = 0
            while i < len(il):
                inst = il[i]
                si = inst.sync_info
                if si is not None and si.on_wait and len(si.on_wait) > 1:
                    waits = list(si.on_wait)
                    for j, w in enumerate(waits[:-1]):
                        nop = mybir.InstNoOp(
                            name=f"{inst.name}-hw{j}",
                            sync_info=mybir.SyncInfo(on_wait=[w], on_update=[]),
                            bass_nofuse=True,
                            engine=inst.engine,
                        )
                        il.insert(i, nop)
                        i += 1
                    si.on_wait = waits[-1:]
                i += 1


def _classify_mask(mask2d):
    """Classify 128x128 blocks of the [S, S] bool mask (query s, key t).

    Returns (status[j][i], patterns) in scores-transposed coords:
    j = key(t) block, i = query(s) block. status: -1 skip, -2 full,
    >=0 index into patterns (multiplicative bf16 0/1 [t, s] blocks).
    """
    status = [[-1] * NB for _ in range(NB)]
    patterns = []
    pat_idx = {}
    for j in range(NB):
        for i in range(NB):
            blk = mask2d[i * 128:(i + 1) * 128, j * 128:(j + 1) * 128]  # [s, t]
            if blk.all():
                status[j][i] = -2
            elif not blk.any():
                status[j][i] = -1
            else:
                mul = np.where(blk.T, 1.0, 0.0).astype(BF)  # [t, s]
                key = mul.tobytes()
                if key not in pat_idx:
                    pat_idx[key] = len(patterns)
                    patterns.append(mul)
                status[j][i] = pat_idx[key]
    return status, patterns


def _plan_chunks(status, patterns):
    """Per (c, j): suffix run of non-skip query blocks within chunk c.

    Returns plan[c][j] = (w, mixes) where w = run width and mixes =
    [(col_offset_in_region, pattern_id), ...] for mixed blocks. Also
    first_j[c]. Asserts the suffix-nested structure the kernel relies on.
    """
    plan = [[None] * NB for _ in range(NCH)]
    first_j = [None] * NCH
    for c in range(NCH):
        i_lo, i_hi = 4 * c, 4 * c + 4
        prev_w = None
        for j in range(NB):
            sts = [status[j][i] for i in range(i_lo, i_hi)]
            nz = [k for k, s in enumerate(sts) if s != -1]
            if not nz:
                plan[c][j] = (0, [])
                continue
            # must be a contiguous suffix of the chunk
            if nz != list(range(nz[0], 4)):
                raise NotImplementedError("mask block structure not suffix-contiguous")
            w = 128 * len(nz)
            if prev_w is not None and w > prev_w:
                raise NotImplementedError("mask runs not nested over key blocks")
            prev_w = w
            mixes = [((k - nz[0]) * 128, sts[k]) for k in nz if sts[k] >= 0]
            plan[c][j] = (w, mixes)
            if first_j[c] is None:
                first_j[c] = j
    return plan, first_j


def _pack_js(plan, c):
    """Greedy-pack consecutive j runs so one st tile / one exp covers
    several j blocks.  Each pack's total 2w must fit 1024 f32 (4KB)."""
    js = [j for j in range(NB) if plan[c][j][0] > 0]
    packs = []
    cur, cur_sz = [], 0
    for j in js:
        sz = 2 * plan[c][j][0]
        if cur and cur_sz + sz > 1024:
            packs.append(cur)
            cur, cur_sz = [], 0
        cur.append(j)
        cur_sz += sz
    if cur:
        packs.append(cur)
    return packs


def _build(plan, first_j, nmix, has_bk, has_bv, has_bo):
    nc = bass.Bass()
    qt = nc.dram_tensor("qt", [128, 4, N], BF16, kind="ExternalInput")
    kt = nc.dram_tensor("kt", [128, 4, N], BF16, kind="ExternalInput")
    vt = nc.dram_tensor("vt", [128, 4, N], BF16, kind="ExternalInput")
    wkt = nc.dram_tensor("wkt", [4, 128, 4, 128], BF16, kind="ExternalInput")
    wvt = nc.dram_tensor("wvt", [128, 4, D], BF16, kind="ExternalInput")
    wot = nc.dram_tensor("wot", [128, 4, D], BF16, kind="ExternalInput")
    bk32 = nc.dram_tensor("bk32", [128, 4], F32, kind="ExternalInput")
    bvb = nc.dram_tensor("bvb", [1, D], BF16, kind="ExternalInput")
    bob = nc.dram_tensor("bob", [1, D], BF16, kind="ExternalInput")
    mixmul = nc.dram_tensor("mixmul", [max(nmix, 1), 128, 128], BF16,
                            kind="ExternalInput")
    out = nc.dram_tensor("out", [N, D], F32, kind="ExternalOutput")

    with tile.TileContext(nc) as tc:
        with ExitStack() as ctx:
            sing = ctx.enter_context(tc.tile_pool(name="sing", bufs=1))
            expp = ctx.enter_context(tc.tile_pool(name="expp", bufs=21))
            rcp = ctx.enter_context(tc.tile_pool(name="rcp", bufs=4))
            ctp = ctx.enter_context(tc.tile_pool(name="ctp", bufs=2))
            cti = ctx.enter_context(tc.tile_pool(name="cti", bufs=4))
            outp = ctx.enter_context(tc.tile_pool(name="outp", bufs=4))
            stp = ctx.enter_context(tc.tile_pool(name="stp", bufs=2, space="PSUM"))
            avp = ctx.enter_context(tc.tile_pool(name="avp", bufs=2, space="PSUM"))
            shp = ctx.enter_context(tc.tile_pool(name="shp", bufs=2, space="PSUM"))

            # ---- input loads: critical-path first.  Attention-phase DMAs
            # go on sync; startup loads spread over scalar/sync/gpsimd.
            # kt0 on sync and wkt on scalar so the first projection's two
            # inputs stream through HWDGE back-to-back instead of serially
            kt_c, qt_c, vt_c = [], [], []
            kt_c = [None] * 4
            qt_c = [None] * 4
            vt_c = [None] * 4
            wkt_ob = []
            for ob in range(4):
                t = sing.tile([128, 4, 128], BF16, tag=f"wktob{ob}",
                              name=f"wktob{ob}")
                wkt_ob.append(t)
            # arrival order follows the iteration schedule
            # b0c0 -> b0c1 -> b1c1 -> b1c0: ch 0, 1, 3, 2
            ch_order = (0, 1, 3, 2)
            for ci, ch in enumerate(ch_order):
                csl = slice(ch * 512, ch * 512 + 512)
                t = sing.tile([128, 4, 512], BF16, tag=f"ktc{ch}",
                              name=f"ktc{ch}")
                nc.sync.dma_start(out=t, in_=kt[:, :, csl])
                kt_c[ch] = t
                if ci == 0:
                    nc.sync.dma_start(out=wkt_ob[0], in_=wkt[0, :, :, :])
                t = sing.tile([128, 4, 512], BF16, tag=f"qtc{ch}",
                              name=f"qtc{ch}")
                nc.scalar.dma_start(out=t, in_=qt[:, :, csl])
                qt_c[ch] = t
                if ci == 0:
                    for ob in range(1, 4):
                        nc.scalar.dma_start(out=wkt_ob[ob],
                                            in_=wkt[ob, :, :, :])
                    wvt_sb = sing.tile([128, 4, D], BF16)
                    nc.scalar.dma_start(out=wvt_sb, in_=wvt[:, :, :])
                else:
                    # v loads ride the scalar HWDGE queue behind the more
                    # critical kq inputs (per-engine priority order holds;
                    # Pool SWDGE DGEs would jump the shared DMA engines)
                    pch = ch_order[ci - 1]
                    t = sing.tile([128, 4, 512], BF16, tag=f"vtc{pch}",
                                  name=f"vtc{pch}")
                    nc.scalar.dma_start(out=t, in_=vt[:, :, pch * 512:pch * 512 + 512])
                    vt_c[pch] = t
            t = sing.tile([128, 4, 512], BF16, tag="vtc2f", name="vtc2f")
            nc.scalar.dma_start(out=t, in_=vt[:, :, 2 * 512:2 * 512 + 512])
            vt_c[2] = t
            mix_sb = sing.tile([128, max(nmix, 1), 128], BF16)
            nc.scalar.dma_start(out=mix_sb, in_=mixmul.rearrange("m t s -> t m s"))
            wot_sb = sing.tile([128, 4, D], BF16)
            nc.scalar.dma_start(out=wot_sb, in_=wot[:, :, :])
            bk_sb = None
            if has_bk:
                bk_sb = sing.tile([128, 4], F32)
                nc.sync.dma_start(out=bk_sb, in_=bk32[:, :])
            bvb_sb = bob_sb = ones_k1 = None
            if has_bv or has_bo:
                ones_k1 = sing.tile([1, 128], BF16)
                nc.vector.memset(ones_k1, 1.0)
            if has_bv:
                bvb_sb = sing.tile([1, D], BF16)
                nc.sync.dma_start(out=bvb_sb, in_=bvb[:, :])
            if has_bo:
                bob_sb = sing.tile([1, D], BF16)
                nc.sync.dma_start(out=bob_sb, in_=bob[:, :])

            kp_sb = sing.tile([128, 4, N], BF16)
            qp_sb = sing.tile([128, 4, N], BF16)
            vp_sb = sing.tile([128, N // 128, 520], BF16)

            # ones columns of vp (denominator trick)
            for nt in range(N // 128):
                nc.vector.memset(
                    vp_sb[:, nt, :].rearrange("p (h u) -> p h u", u=65)[:, :, 64:65],
                    1.0)

            fillers = []
            staged = []    # (pop_stamp, thunk): finals wait out their
                           # transpose latency before becoming poppable
            pop_ctr = [0]
            copy_rr = [0]

            def psum_copy(dst, src):
                # PSUM->SBUF copies on DVE; ACT stays exp-only and GPSIMD
                # cannot access PSUM
                nc.vector.tensor_copy(out=dst, in_=src)

            def kq_half(ob, ch, which):
                csl = slice(ch * 512, ch * 512 + 512)
                src = kt_c[ch] if which == "k" else qt_c[ch]
                dst = kp_sb if which == "k" else qp_sb
                ps = shp.tile([128, 512], F32, tag="sh", name=f"ps{which}")
                for db in range(4):
                    nc.tensor.matmul(
                        ps, wkt_ob[ob][:, db, :],
                        src[:, db, :], start=(db == 0), stop=(db == 3))
                if has_bk:
                    if which == "k":
                        nc.scalar.add(dst[:, ob, csl], ps, bk_sb[:, ob:ob + 1])
                    else:
                        nc.vector.tensor_scalar_add(
                            dst[:, ob, csl], ps, bk_sb[:, ob:ob + 1])
                else:
                    psum_copy(dst[:, ob, csl], ps)

            def kq_group(ob, ch):
                kq_half(ob, ch, "k")
                kq_half(ob, ch, "q")

            def v_proj(nt):
                psV = shp.tile([128, 512], F32, tag="sh")
                for db in range(4):
                    nc.tensor.matmul(
                        psV, vt_c[nt // 4][:, db, (nt % 4) * 128:(nt % 4) * 128 + 128],
                        wvt_sb[:, db, :], start=(db == 0),
                        stop=(db == 3 and not has_bv))
                if has_bv:
                    nc.tensor.matmul(psV, ones_k1, bvb_sb[0:1, :],
                                     start=False, stop=True)
                dst = vp_sb[:, nt, :].rearrange("p (h u) -> p h u", u=65)[:, :, 0:64]
                src = psV.rearrange("p (h u) -> p h u", u=64)
                psum_copy(dst, src)

            def pop_filler(k=1):
                for _ in range(k):
                    pop_ctr[0] += 1
                    while staged and staged[0][0] + 16 <= pop_ctr[0]:
                        fillers.append((("fin",), staged.pop(0)[1]))
                    if fillers:
                        fillers.pop(0)[1]()

            def need_filler(key):
                """Force-emit a specific filler now (dependency deadline)."""
                for fi, (k, thunk) in enumerate(fillers):
                    if k == key:
                        fillers.pop(fi)
                        thunk()
                        return

            ct_t_tiles = {}

            def attention_packs(b, hp, c, ex_t):
                """Thunks: scores+exp+mask, one per pack of j blocks."""
                packs = _pack_js(plan, c)
                thunks = []

                def do_pack(pack):
                    # PSUM bank rule: each matmul output must stay inside one
                    # 2KB bank.  h0 segments stack downward from col 512
                    # (bank 0), h1 segments upward from col 512 (bank 1); the
                    # exp covers the contiguous union [512-tw, 512+tw).
                    tw = sum(plan[c][j][0] for j in pack)
                    st = stp.tile([128, 1024], F32, tag="st")
                    ex = expp.tile([128, 1024], BF16, tag="ex")
                    pre = 0
                    for j in pack:
                        w, _ = plan[c][j]
                        tsl = slice(b * S + j * 128, b * S + j * 128 + 128)
                        ssl = slice(b * S + c * 512 + 512 - w,
                                    b * S + c * 512 + 512)
                        h0s = 512 - pre - w
                        h1s = 512 + pre
                        nc.tensor.matmul(st[:, h0s:h0s + w],
                                         kp_sb[0:64, hp, tsl],
                                         qp_sb[0:64, hp, ssl],
                                         start=True, stop=True)
                        nc.tensor.matmul(st[:, h1s:h1s + w],
                                         kp_sb[64:128, hp, tsl],
                                         qp_sb[64:128, hp, ssl],
                                         start=True, stop=True)
                        ex_t[j] = (ex, h0s, h1s, w)
                        pre += w
                    nc.scalar.activation(
                        ex[:, 512 - tw:512 + tw], st[:, 512 - tw:512 + tw],
                        mybir.ActivationFunctionType.Exp, scale=0.125)
                    # 0/1 mask multiply for mixed blocks (per head half)
                    for j in pack:
                        w, mixes = plan[c][j]
                        _, h0s, h1s, _ = ex_t[j]
                        for moff, pid in mixes:
                            for hs in (h0s, h1s):
                                sl = ex[:, hs + moff:hs + moff + 128]
                                nc.vector.tensor_mul(
                                    sl, sl, mix_sb[:, pid, :])

                for pack in packs:
                    import functools
                    thunks.append(functools.partial(do_pack, pack))
                return thunks

            def attention_avs(b, hp, c, ex_t, last_hp):
                """Thunks: AV + normalize, one per query block i (swapped
                operands: ex stationary, vp moving)."""
                h0 = 2 * hp
                thunks = []

                def do_av(i):
                    js_i = []
                    for j, (ex, h0s, h1s, w) in ex_t.items():
                        i_start = 4 * c + 4 - w // 128
                        if i >= i_start:
                            o = (i - i_start) * 128
                            js_i.append((j, ex, (h0s + o, h1s + o)))
                    if not js_i:
                        return
                    for j, _, _ in js_i:
                        need_filler(("vp", b * NB + j))
                    av = avp.tile([128, 2, 65], F32, tag="av")
                    nmm = len(js_i) * 2
                    mi = 0
                    for j, ex, hss in js_i:
                        vrow = b * NB + j
                        for h in range(2):
                            # single accumulation group per av tile: PSUM
                            # zeroing is bank-granular (start marks the whole
                            # bank pending-zero; first write to each address
                            # assigns, later writes accumulate)
                            nc.tensor.matmul(
                                av[:, h, :],
                                ex[:, hss[h]:hss[h] + 128],
                                vp_sb[:, vrow,
                                      65 * (h0 + h):65 * (h0 + h) + 65],
                                start=(mi == 0), stop=(mi == nmm - 1),
                                skip_group_check=True)
                            mi += 1
                    # normalize: per-partition reciprocal + broadcast mul
                    key = (b, i)
                    if key not in ct_t_tiles:
                        ct_t_tiles[key] = ctp.tile([128, 512], BF16,
                                                   name=f"ctt{b}_{i}",
                                                   tag=f"ctt{b}_{i % 4}")
                    ct_t = ct_t_tiles[key]
                    rc = rcp.tile([128, 2], BF16, tag="rc")
                    with nc.allow_low_precision(reason="softmax recip bf16"):
                        nc.vector.reciprocal(out=rc, in_=av[:, :, 64])
                    dst = ct_t[:, 128 * hp:128 * hp + 128].rearrange(
                        "p (h w) -> p h w", h=2)
                    nc.vector.tensor_mul(
                        dst, av[:, :, 0:64],
                        rc[:, :, None].broadcast_to([128, 2, 64]))
                    if last_hp:
                        finish_block(b, i, ct_t)

                import functools
                for i in range(4 * c, 4 * c + 4):
                    thunks.append(functools.partial(do_av, i))
                return thunks

            def finish_block(b, i, ct_t):
                """transpose ct_t -> feature-major, then queue out-proj."""
                ct_i = cti.tile([128, 4, 128], BF16, tag="cti")
                nc.sync.dma_start_transpose(ct_i[:, :, :], ct_t[:, :])
                del ct_t_tiles[(b, i)]

                def final(b=b, i=i, ct_i=ct_i):
                    psO = shp.tile([128, 512], F32, tag="sh")
                    for db in range(4):
                        nc.tensor.matmul(
                            psO, ct_i[:, db, :], wot_sb[:, db, :],
                            start=(db == 0), stop=(db == 3 and not has_bo))
                    if has_bo:
                        nc.tensor.matmul(psO, ones_k1, bob_sb[0:1, :],
                                         start=False, stop=True)
                    ot = outp.tile([128, 512], F32)
                    psum_copy(ot, psO)
                    row = b * S + i * 128
                    nc.sync.dma_start(out=out[row:row + 128, :], in_=ot)

                staged.append((pop_ctr[0], final))

            # ---- emission schedule (software-pipelined) ----
            # upfront: only what iteration 0's scores need; the rest of the
            # projections become ordered fillers consumed during attention.
            import functools
            # upfront: iteration 0 (b0, hp0, c0) needs only ob0/ch0
            kq_half(0, 0, "k")
            kq_half(0, 0, "q")
            # deadline-ordered fillers matching the b0c0,b0c1,b1c1,b1c0 seq;
            # keys let consumers force-emit their prerequisites in time
            def FK(ob, ch):
                fillers.append((("kq", ob, ch), functools.partial(kq_group, ob, ch)))

            def FV(nt):
                fillers.append((("vp", nt), functools.partial(v_proj, nt)))

            FK(1, 0)
            FK(2, 0)
            FV(0)
            FV(1)
            FK(3, 0)
            FV(2)
            FV(3)
            FK(0, 1)
            FK(1, 1)
            FV(4)
            FV(5)
            FK(2, 1)
            FV(6)
            FV(7)
            FK(3, 1)
            for ob in range(4):
                FK(ob, 3)
                FK(ob, 2)
            for nt in range(12, 16):
                FV(nt)
            for nt in range(8, 12):
                FV(nt)

            seq = []
            border = {0: (0, 1), 1: (1, 0)}
            for b in range(BL):
                for c in border[b % 2] if NCH == 2 else range(NCH):
                    for hp in range(4):
                        seq.append((b, hp, c))

            # iteration k's AV phase is interleaved with iteration k+2's
            # scores/exp packs (2-deep software pipeline): by the time an AV
            # runs, its exps retired during iteration k+1, so PE never waits
            # on ACT across iteration boundaries
            pend = []      # queue of AV thunk lists
            nseq = len(seq)
            for it, (b, hp, c) in enumerate(seq):
                # scores need this iteration's kq projections emitted first
                for ch in ([2 * b] if c == 0 else [2 * b, 2 * b + 1]):
                    need_filler(("kq", hp, ch))
                ex_t = {}
                packs = attention_packs(b, hp, c, ex_t)
                avs = attention_avs(b, hp, c, ex_t, last_hp=(hp == 3))
                ready = pend.pop(0) if (len(pend) >= 2 or
                                        (pend and it == nseq - 1)) else []
                pops = 0
                # cap pops in the first (b0) half so fillers remain for the
                # ACT-bound b1c1 phase; first iterations also delay pops so
                # a not-yet-loaded filler can't head-of-line block PE
                cap = 3 if it < nseq // 2 else (4 if it < 3 * nseq // 4 else 99)
                for x in range(max(len(packs), len(ready))):
                    do_pop = (it >= 2 or x >= 2) and pops < cap
                    if x < len(packs):
                        packs[x]()
                        if do_pop:
                            pop_filler(1)
                            pops += 1
                    if x < len(ready):
                        ready[x]()
                        if do_pop and pops < cap:
                            pop_filler(1)
                            pops += 1
                pend.append(avs)
            for avs in pend:
                for av in avs:
                    av()
                    pop_filler(1)
            while fillers or staged:
                pop_filler(1)

    return nc


_prog_cache = {}


def kernel(q, k, v, mask, zero_pad, Wk, bk, Wv, bv, Wo, bo):
    global LAST_SIM_NS, LAST_EXEC_NS
    q = np.asarray(q, dtype=np.float32)
    k = np.asarray(k, dtype=np.float32)
    v = np.asarray(v, dtype=np.float32)
    Wk = np.asarray(Wk, dtype=np.float32)
    Wv = np.asarray(Wv, dtype=np.float32)
    Wo = np.asarray(Wo, dtype=np.float32)
    bk = np.asarray(bk, dtype=np.float32).reshape(D)
    bv = np.asarray(bv, dtype=np.float32).reshape(D)
    bo = np.asarray(bo, dtype=np.float32).reshape(D)
    mask2d = np.asarray(mask).reshape(S, S).astype(bool)
    zp = int(np.asarray(zero_pad))

    status, patterns = _classify_mask(mask2d)
    plan, first_j = _plan_chunks(status, patterns)
    nmix = len(patterns)
    has_bk = bool(np.any(bk))
    has_bv = bool(np.any(bv))
    has_bo = bool(np.any(bo))

    sig = (tuple(tuple(r) for r in status), nmix, has_bk, has_bv, has_bo)
    if sig not in _prog_cache:
        nc_new = _build(plan, first_j, nmix, has_bk, has_bv, has_bo)
        legalize_waits(nc_new)   # hardware-only pass (sim runs pre-legalized)
        _prog_cache[sig] = nc_new
    nc = _prog_cache[sig]

    def _sbuf_layout(wt):
        # [D, X] -> [128, 4, X]: row d = a*128+p  ->  [p, a, :]
        return np.ascontiguousarray(wt.reshape(4, 128, -1).transpose(1, 0, 2))

    # wkt grouped by ob block: [4, 128, 4, 128], wkt[ob][p, db, c] =
    # Wk.T[db*128+p, ob*128+c]
    wkt = np.ascontiguousarray(
        _sbuf_layout(Wk.T.astype(BF)).reshape(128, 4, 4, 128)
        .transpose(2, 0, 1, 3))
    wvt = _sbuf_layout(Wv.T.astype(BF))
    wot = _sbuf_layout(Wo.T.astype(BF))
    bk32 = np.ascontiguousarray(bk.reshape(4, 128).T).astype(np.float32)
    bvb = bv.reshape(1, D).astype(BF)
    bob = bo.reshape(1, D).astype(BF)
    mixmul = (np.stack(patterns) if patterns
              else np.zeros((1, 128, 128), np.float32)).astype(BF)

    common = dict(wkt=wkt, wvt=wvt, wot=wot, bk32=bk32, bvb=bvb, bob=bob,
                  mixmul=mixmul)
    in_maps = []
    for ci in range(NCORES):
        sl = slice(ci * BL, (ci + 1) * BL)
        in_maps.append(dict(
            qt=_sbuf_layout(q[sl].reshape(N, D).T.astype(BF)),
            kt=_sbuf_layout(k[sl].reshape(N, D).T.astype(BF)),
            vt=_sbuf_layout(v[sl].reshape(N, D).T.astype(BF)),
            **common))

    if os.environ.get("BASS_KERNEL_SIM_TIME"):
        from concourse.timeline_sim import TimelineSim
        LAST_SIM_NS = TimelineSim(nc).simulate()

    res = run_bass_kernel_spmd(nc, in_maps, list(range(NCORES)))
    LAST_EXEC_NS = res.exec_time_ns

    outs = [res.results[ci]["out"].reshape(BL, S, D) for ci in range(NCORES)]
    full = np.concatenate(outs, axis=0)
    if zp:
        full[:, 0, :] = bo
    return full
